# revision 3
# baseline (speedup 1.0000x reference)
"""Trainium2 Bass kernel for nn_CustomLoss: sum((predicted - target)**2) / 2.

Data-parallel across 8 NeuronCores: rows are sharded, each core streams its
128 MiB shard through SBUF and computes per-partition partial sums of
squared differences; the host sums the 8x128xNSEQ partials and halves.

Raw Bass (not Tile): the walrus codegen on this path allows only one sync
wait per compute instruction, so sync is explicit standalone wait_ge's.

Pipeline per core:
  SP ring   : pred DMAs (HWDGE queue 1)
  ACT ring  : targ DMAs (HWDGE queue 2, interleaved with squares)
  DVE       : diff = pred - targ (in place over pred)
  ACT       : square(diff) in place + per-partition accumulate -> acc[:, seq]

Tiling: 15 full tiles of 4 MiB/tensor double-buffered across 2 slots, then
a 2 MiB slotted half tile, then two 1 MiB chunks with dedicated single-use
buffers and one semaphore per DMA. The tail chunks need no gating, so every
DMA trigger is enqueued long before the rings reach it and the 128 MiB
streams back to back; the only exposed latency is ~6 us of runtime preamble
and the last chunk's subtract+square (~5 us).

The Bass-init all-engine barrier is suppressed: its only purpose is
ordering the Pool const-AP memsets against consumers, and this kernel uses
an ACT-local memzero'd bias tile instead of the const APs.

Self-contained: hardcodes shapes from the problem spec; only depends on the
container's bass/concourse install at /opt/trn_rl_repo.
"""

import sys

if "/opt/trn_rl_repo" not in sys.path:
    sys.path.insert(0, "/opt/trn_rl_repo")

import numpy as np

N, D = 1048576, 128
NCORES = 8
ELEMS_PER_CORE = (N // NCORES) * D  # 16,777,216 fp32 = 64 MiB per tensor
P = 128                    # SBUF partitions
FTOT = ELEMS_PER_CORE // P  # 131072 fp32 per partition per tensor
FBIG = 8192                # full tile: 4 MiB per tensor per DMA

# (dram col offset, width, slot or None) per pipeline iteration; slot=None
# means a dedicated chunk buffer. Slots alternate 0/1 for the slotted seqs.
SEQS = [(i * FBIG, FBIG) for i in range(15)] + [(122880, 4096)]
SEQS = [(off, w, i % 2) for i, (off, w) in enumerate(SEQS)]
SEQS += [(126976, 2048, None), (129024, 2048, None)]
NSEQ = len(SEQS)
NSLOTTED = 16
CHUNKS = [i for i, s in enumerate(SEQS) if s[2] is None]

# Set by test harness to capture a HW profile; harness-default is plain run.
TRACE = False
LAST_EXEC_NS = None
LAST_RESULT = None

_cached_nc = None


def _build():
    from contextlib import ExitStack

    from concourse import bass, mybir

    # Suppress the Bass-init all-engine barrier (see module docstring).
    orig_barrier = bass.Bass.all_engine_barrier
    bass.Bass.all_engine_barrier = lambda self, *a, **k: None
    try:
        nc = bass.Bass()
    finally:
        bass.Bass.all_engine_barrier = orig_barrier

    f32 = mybir.dt.float32
    pred_ext = nc.declare_dram_parameter("predicted", [P, FTOT], f32, isOutput=False)
    targ_ext = nc.declare_dram_parameter("target", [P, FTOT], f32, isOutput=False)
    out_ext = nc.declare_dram_parameter("partials", [P, NSEQ], f32, isOutput=True)

    ctx = ExitStack()
    # one sem per chunk DMA: chunk DMAs are concurrently in flight, so a
    # shared counting sem would be unsound (the total can reach the target
    # while one transfer is still partial)
    cp_p, cp_t, pred_c, targ_c = {}, {}, {}, {}
    for seq in CHUNKS:
        w = SEQS[seq][1]
        cp_p[seq] = ctx.enter_context(nc.semaphore(f"cp_p{seq}"))
        cp_t[seq] = ctx.enter_context(nc.semaphore(f"cp_t{seq}"))
        pred_c[seq] = ctx.enter_context(nc.sbuf_tensor(f"pred_c{seq}", [P, w], f32))
        targ_c[seq] = ctx.enter_context(nc.sbuf_tensor(f"targ_c{seq}", [P, w], f32))

    with (
        ctx,
        nc.semaphore("psem_a") as psem_a,
        nc.semaphore("psem_b") as psem_b,
        nc.semaphore("tsem_a") as tsem_a,
        nc.semaphore("tsem_b") as tsem_b,
        nc.semaphore("dve_sem") as dve_sem,
        nc.semaphore("act_sem") as act_sem,
        nc.semaphore("out_sem") as out_sem,
        nc.sbuf_tensor("pred_a", [P, FBIG], f32) as pred_a,
        nc.sbuf_tensor("pred_b", [P, FBIG], f32) as pred_b,
        nc.sbuf_tensor("targ_a", [P, FBIG], f32) as targ_a,
        nc.sbuf_tensor("targ_b", [P, FBIG], f32) as targ_b,
        nc.sbuf_tensor("zbias", [P, 1], f32) as zbias,
        nc.sbuf_tensor("acc", [P, NSEQ], f32) as acc,
        nc.Block() as block,
    ):
        pred_t = [pred_a, pred_b]
        targ_t = [targ_a, targ_b]
        psem = [psem_a, psem_b]
        tsem = [tsem_a, tsem_b]
        # per-slot occurrence index of each slotted seq, for sem counting
        slot_ord = {}
        cnt = [0, 0]
        for seq, (_, _, s) in enumerate(SEQS):
            if s is not None:
                cnt[s] += 1
                slot_ord[seq] = cnt[s]

        def pred_ap(seq):
            off, w, s = SEQS[seq]
            return pred_t[s][:, 0:w] if s is not None else pred_c[seq][:]

        def targ_ap(seq):
            off, w, s = SEQS[seq]
            return targ_t[s][:, 0:w] if s is not None else targ_c[seq][:]

        def targ_dma(eng, seq):
            off, w, s = SEQS[seq]
            sem = tsem[s] if s is not None else cp_t[seq]
            eng.dma_start(
                out=targ_ap(seq), in_=targ_ext[:, off : off + w]
            ).then_inc(sem, 16)

        @block.sync
        def _(sync):
            for seq, (off, w, s) in enumerate(SEQS):
                if s is not None and seq >= 2:
                    # slot reused: the square of the previous tenant (last
                    # reader and in-place writer) must be done
                    sync.wait_ge(act_sem, seq - 1)
                sync.dma_start(
                    out=pred_ap(seq), in_=pred_ext[:, off : off + w]
                ).then_inc(psem[s] if s is not None else cp_p[seq], 16)
            sync.wait_ge(act_sem, NSEQ)
            sync.dma_start(out=out_ext[:], in_=acc[:]).then_inc(out_sem, 16)
            sync.wait_ge(out_sem, 16)

        @block.vector
        def _(vector):
            for seq, (off, w, s) in enumerate(SEQS):
                if s is not None:
                    vector.wait_ge(psem[s], 16 * slot_ord[seq])
                    vector.wait_ge(tsem[s], 16 * slot_ord[seq])
                else:
                    vector.wait_ge(cp_p[seq], 16)
                    vector.wait_ge(cp_t[seq], 16)
                vector.tensor_sub(
                    out=pred_ap(seq), in0=pred_ap(seq), in1=targ_ap(seq)
                ).then_inc(dve_sem, 1)

        @block.scalar
        def _(scalar):
            # zero bias for Square, owned by ACT itself (program order makes
            # it visible to every square; avoids the framework const APs and
            # therefore any dependence on the suppressed init barrier)
            scalar.memzero(zbias[:])
            # targ DMAs ride the ACT HWDGE ring, interleaved with the
            # squares; full-tile slot-reuse safety is ACT program order (the
            # square of the previous tenant precedes each trigger), chunk
            # buffers are single-use and need no gating.
            targ_dma(scalar, 0)
            targ_dma(scalar, 1)
            for seq in range(NSEQ):
                scalar.wait_ge(dve_sem, seq + 1)
                # square(diff) in place + row-sum. In-place is safe: the
                # next writer of this region is a pred DMA gated on act_sem
                # (cross-engine sem => writes drained), never a DMA
                # triggered by ACT itself right after.
                scalar.activation(
                    out=pred_ap(seq),
                    in_=pred_ap(seq),
                    func=mybir.ActivationFunctionType.Square,
                    bias=zbias[:],
                    accum_out=acc[:, seq : seq + 1],
                ).then_inc(act_sem, 1)
                nxt = seq + 2
                if nxt < NSLOTTED:
                    targ_dma(scalar, nxt)
                elif nxt == NSLOTTED:  # last slotted targ sent; queue chunks
                    for j in CHUNKS:
                        targ_dma(scalar, j)

    return nc


def kernel(predicted, target):
    global _cached_nc, LAST_EXEC_NS, LAST_RESULT
    from concourse.bass_utils import run_bass_kernel_spmd

    if _cached_nc is None:
        _cached_nc = _build()
    nc = _cached_nc

    p = np.ascontiguousarray(np.asarray(predicted, dtype=np.float32)).reshape(
        NCORES, P, FTOT
    )
    t = np.ascontiguousarray(np.asarray(target, dtype=np.float32)).reshape(
        NCORES, P, FTOT
    )
    in_maps = [{"predicted": p[c], "target": t[c]} for c in range(NCORES)]
    res = run_bass_kernel_spmd(nc, in_maps, list(range(NCORES)), trace=TRACE)
    LAST_EXEC_NS = res.exec_time_ns
    LAST_RESULT = res
    total = sum(r["partials"].sum(dtype=np.float64) for r in res.results)
    return np.float32(total / 2.0)

